# revision 31
# baseline (speedup 1.0000x reference)
"""Trainium2 Bass kernel for nn_DecoderBlock (S=4096, D=768, H=12).

Strategy (8 NeuronCores, SPMD):
  - Sequence-parallel: core c owns rows [c*512, (c+1)*512) of the sequence.
  - All activations kept in transposed layout (feature dim on partitions,
    sequence on the free axis).  LayerNorm statistics are computed with
    ones-vector matmuls (partition-axis reduction on the PE), so no on-chip
    transposes are needed anywhere.
  - QKV/FFN weights are host-transposed and cast to bf16; matmuls run in
    bf16 with fp32 PSUM accumulation.
  - K (transposed) and natural-layout V are exchanged with chunked
    AllGather collectives (bf16) so attention can start while later chunks
    are still in flight.
  - Attention processes a head PAIR per sk-tile step: the two score
    matmuls (K=64) are row-packed into disjoint PE row groups, one
    [128,1024] Exp covers both heads, the two PV matmuls are col-packed
    (tile_position (0,0)/(0,64)) into one PSUM bank, and the softmax
    denominators come from M=1 ones-matmuls packed at (0,0)/(0,32).
    Softmax skips the max-subtraction (scores*scale is bounded ~3 for
    this distribution); 1/x is computed as exp(-ln(x)) on the scalar
    engine (vector reciprocal is 8 cyc/elem).
"""

import os
import sys

for _p in ("/opt/trn_rl_repo", os.path.expanduser("~/.axon_site/_ro/trn_rl_repo")):
    if os.path.isdir(_p) and _p not in sys.path:
        sys.path.append(_p)

import numpy as np
from contextlib import ExitStack

import concourse.bass as bass
import concourse.tile as tile
from concourse import bacc, mybir

F32 = mybir.dt.float32
BF16 = mybir.dt.bfloat16
AF = mybir.ActivationFunctionType
ALU = mybir.AluOpType


class Cfg:
    def __init__(self, S=4096, D=768, H=12, NC=8, G=2, eps=1e-5):
        self.S, self.D, self.H, self.NC, self.eps = S, D, H, NC, eps
        self.G = G                 # allgather chunks
        self.DH = D // H
        assert self.DH == 64 and H % 2 == 0 and D % 128 == 0
        self.SL = S // NC          # local sequence rows per core
        assert self.SL % 128 == 0 and self.SL <= 512
        self.ND = D // 128         # d tiles
        self.HP = H // 2           # head pairs
        self.NSK = S // 128        # sk tiles (global)
        self.NFF = 4 * D // 128    # ffn hidden tiles
        assert self.ND % G == 0 and self.HP % G == 0


def _vchunks(D):
    n = (D + 511) // 512
    assert D % n == 0
    return D // n


def build(cfg: Cfg, debug=False, enable_asserts=False, gelu_compose=False):
    nc = bacc.Bacc(
        "TRN2",
        target_bir_lowering=False,
        debug=debug,
        enable_asserts=enable_asserts,
        num_devices=cfg.NC,
    )
    S, D, H, SL, G = cfg.S, cfg.D, cfg.H, cfg.SL, cfg.G
    ND, HP, NSK, NFF, NC = cfg.ND, cfg.HP, cfg.NSK, cfg.NFF, cfg.NC
    scale = 1.0 / float(np.sqrt(cfg.DH))

    # ---- DRAM I/O ----------------------------------------------------------
    xT = nc.dram_tensor("xT", [D, SL], F32, kind="ExternalInput").ap()
    w_qkT = nc.dram_tensor("w_qkT", [D, 2 * D], BF16, kind="ExternalInput").ap()
    w_vT = nc.dram_tensor("w_vT", [D, D], BF16, kind="ExternalInput").ap()
    b_qk = nc.dram_tensor("b_qk", [128, 2 * D // 128], F32, kind="ExternalInput").ap()
    b_v = nc.dram_tensor("b_v", [1, D], F32, kind="ExternalInput").ap()
    ln1w = nc.dram_tensor("ln1w", [128, ND], F32, kind="ExternalInput").ap()
    ln1b = nc.dram_tensor("ln1b", [128, ND], F32, kind="ExternalInput").ap()
    ln2w = nc.dram_tensor("ln2w", [128, ND], F32, kind="ExternalInput").ap()
    ln2b = nc.dram_tensor("ln2b", [128, ND], F32, kind="ExternalInput").ap()
    w_fcT = nc.dram_tensor("w_fcT", [D, 4 * D], BF16, kind="ExternalInput").ap()
    b_fc = nc.dram_tensor("b_fc", [128, NFF], F32, kind="ExternalInput").ap()
    w_projT = nc.dram_tensor("w_projT", [4 * D, D], BF16, kind="ExternalInput").ap()
    b_proj = nc.dram_tensor("b_proj", [128, ND], F32, kind="ExternalInput").ap()
    outT = nc.dram_tensor("outT", [D, SL], F32, kind="ExternalOutput").ap()

    with tile.TileContext(nc) as tc, ExitStack() as top:
        persist = top.enter_context(tc.tile_pool(name="persist", bufs=1))
        dram = top.enter_context(tc.tile_pool(name="dram", bufs=1, space="DRAM"))

        # constants / parameters resident in SBUF
        ones_col = persist.tile([128, 1], F32)
        nc.vector.memset(ones_col[:], 1.0)
        ones_cb = persist.tile([128, 1], BF16)
        nc.vector.memset(ones_cb[:], 1.0)
        ones33 = persist.tile([33, 128], F32)
        nc.vector.memset(ones33[:], 1.0)
        eps_tile = persist.tile([1, 1], F32)
        nc.vector.memset(eps_tile[:], float(cfg.eps))

        # tiny warmup collective: absorbs TOPSP launch latency + core skew
        warm_in = dram.tile([64], BF16)
        warm_out = dram.tile([64 * NC], BF16,
                             addr_space="Shared" if NC > 4 else "Local")
        wsb = persist.tile([1, 64], BF16)
        nc.vector.memset(wsb[:], 1.0)
        nc.sync.dma_start(warm_in[:], wsb[0, :])
        nc.gpsimd.collective_compute(
            "AllGather", ALU.bypass, replica_groups=[list(range(NC))],
            ins=[warm_in[:]], outs=[warm_out[:]])

        b_qk_sb = persist.tile([128, 2 * D // 128], F32)
        nc.sync.dma_start(b_qk_sb[:], b_qk[:])
        ln1w_sb = persist.tile([128, ND], F32)
        nc.sync.dma_start(ln1w_sb[:], ln1w[:])
        ln1b_sb = persist.tile([128, ND], F32)
        nc.sync.dma_start(ln1b_sb[:], ln1b[:])
        ln2w_sb = persist.tile([128, ND], F32)
        nc.sync.dma_start(ln2w_sb[:], ln2w[:])
        ln2b_sb = persist.tile([128, ND], F32)
        nc.sync.dma_start(ln2b_sb[:], ln2b[:])
        b_fc_sb = persist.tile([128, NFF], F32)
        nc.sync.dma_start(b_fc_sb[:], b_fc[:])
        b_proj_sb = persist.tile([128, ND], F32)
        nc.sync.dma_start(b_proj_sb[:], b_proj[:])
        b_v_sb = persist.tile([1, D], F32)
        nc.sync.dma_start(b_v_sb[:], b_v[:])

        # persistent activations
        ln1x = [persist.tile([128, SL], F32, name=f"ln1x{t}") for t in range(ND)]
        ctxu = [persist.tile([128, SL], F32, name=f"ctxu{t}") for t in range(ND)]
        # softmax denominators: even heads on partition 0, odd on partition 32
        srow = persist.tile([33, HP * SL], F32)
        nc.vector.memset(srow[:], 1.0)
        q_sb = [persist.tile([128, SL], BF16, name=f"q_sb{t}") for t in range(HP)]

        def layernorm_T(src_tiles, w_sb, b_sb, out_f32, out_bf16):
            """LayerNorm over the partition (feature) axis of transposed tiles."""
            with tc.tile_pool(name="ln_ps", bufs=1, space="PSUM") as lps, \
                 tc.tile_pool(name="ln_sb", bufs=2) as lsb:
                sums = lps.tile([1, SL], F32, tag="st", bufs=2)
                sumsq = lps.tile([1, SL], F32, tag="st", bufs=2)
                sq = [lsb.tile([128, SL], F32, tag="lntmp", bufs=2, name=f"sq{t}")
                      for t in range(ND)]
                for t in range(ND):
                    nc.vector.tensor_tensor(sq[t][:], src_tiles[t][:],
                                            src_tiles[t][:], op=ALU.mult)
                for t in range(ND):
                    nc.tensor.matmul(sums[:], ones_col[:], src_tiles[t][:],
                                     start=(t == 0), stop=(t == ND - 1))
                for t in range(ND):
                    nc.tensor.matmul(sumsq[:], ones_col[:], sq[t][:],
                                     start=(t == 0), stop=(t == ND - 1))
                mean = lsb.tile([1, SL], F32)
                ex2 = lsb.tile([1, SL], F32)
                msq = lsb.tile([1, SL], F32)
                var = lsb.tile([1, SL], F32)
                lnv = lsb.tile([1, SL], F32)
                rstd = lsb.tile([1, SL], F32)
                nc.vector.tensor_scalar_mul(mean[:], sums[:], 1.0 / D)
                nc.vector.tensor_scalar_mul(ex2[:], sumsq[:], 1.0 / D)
                nc.vector.tensor_tensor(msq[:], mean[:], mean[:], op=ALU.mult)
                nc.vector.tensor_tensor(var[:], ex2[:], msq[:], op=ALU.subtract)
                nc.scalar.activation(lnv[:], var[:], AF.Ln, bias=eps_tile[:])
                nc.scalar.activation(rstd[:], lnv[:], AF.Exp, scale=-0.5)
                with tc.tile_pool(name="lnb_ps", bufs=1, space="PSUM") as bps:
                    meanB = bps.tile([128, SL], F32, tag="bc", bufs=2)
                    rstdB = bps.tile([128, SL], F32, tag="bc", bufs=2)
                    nc.tensor.matmul(meanB[:], ones33[0:1, :], mean[:],
                                     start=True, stop=True)
                    nc.tensor.matmul(rstdB[:], ones33[0:1, :], rstd[:],
                                     start=True, stop=True)
                    for t in range(ND):
                        cen = lsb.tile([128, SL], F32, tag="lntmp", bufs=2,
                                       name=f"cen{t}")
                        nc.vector.tensor_tensor(cen[:], src_tiles[t][:],
                                                meanB[:], op=ALU.subtract)
                        nc.vector.tensor_tensor(cen[:], cen[:], rstdB[:],
                                                op=ALU.mult)
                        nc.scalar.activation(out_f32[t][:], cen[:], AF.Identity,
                                             bias=b_sb[:, t:t + 1],
                                             scale=w_sb[:, t:t + 1])
                        nc.vector.tensor_copy(out_bf16[t][:], out_f32[t][:])

        # ==== phase 1: LN1 ====================================================
        p12 = tc.alloc_tile_pool(name="p12", bufs=1)
        ln1xb = [p12.tile([128, SL], BF16, name=f"ln1xb{t}") for t in range(ND)]
        with tc.tile_pool(name="xin", bufs=1) as xin:
            x_sb = [xin.tile([128, SL], F32, name=f"x_sb{t}") for t in range(ND)]
            for t in range(ND):
                nc.sync.dma_start(x_sb[t][:], xT[128 * t:128 * (t + 1), :])
            layernorm_T(x_sb, ln1w_sb, ln1b_sb, ln1x, ln1xb)

        # ==== phase 2: qkv + v, write own k/v to DRAM (chunked by heads) ======
        # chunk g covers feature rows [g*D/G, (g+1)*D/G) of k^T / columns of v;
        # each chunk's k and v ride in ONE flat allgather (k block then v block)
        DG = D // G
        CB = DG * SL   # elements per k (or v) block per chunk
        kv_own = [dram.tile([2 * CB], BF16, name=f"kv_own{g}") for g in range(G)]
        gspace = "Shared" if NC > 4 else "Local"
        kv_gath = [dram.tile([NC * 2 * CB], BF16, addr_space=gspace,
                             name=f"kv_gath{g}") for g in range(G)]
        grp = [list(range(NC))]
        NDG = ND // G   # k j-tiles per chunk

        with tc.tile_pool(name="wqkv", bufs=1) as wp, \
             tc.tile_pool(name="qkv_ps", bufs=1, space="PSUM") as qps, \
             tc.tile_pool(name="kv_sb", bufs=1) as kvp:
            w_qk_sb = [wp.tile([128, 2 * D], BF16, name=f"wqk{t}") for t in range(ND)]
            w_v_sb = [wp.tile([128, D], BF16, name=f"wv{t}") for t in range(ND)]
            for t in range(ND):
                nc.sync.dma_start(w_qk_sb[t][:], w_qkT[128 * t:128 * (t + 1), :])
                nc.sync.dma_start(w_v_sb[t][:], w_vT[128 * t:128 * (t + 1), :])

            # k then v per chunk so allgathers launch early and interleave
            k_sb = [kvp.tile([128, SL], BF16, name=f"k_sb{t}") for t in range(ND)]
            bvb_sb = kvp.tile([128, D], F32)
            with tc.tile_pool(name="bv_ps", bufs=1, space="PSUM") as bvp:
                BC = D // ((D + 511) // 512)
                for i in range(D // BC):
                    bvb = bvp.tile([128, 512], F32, tag="bvb", bufs=2,
                                   name=f"bvb{i}")
                    nc.tensor.matmul(bvb[:, 0:BC], ones33[0:1, :],
                                     b_v_sb[:, BC * i:BC * (i + 1)],
                                     start=True, stop=True)
                    nc.vector.tensor_copy(bvb_sb[:, BC * i:BC * (i + 1)],
                                          bvb[:, 0:BC])
            n_s = SL // 128
            for g in range(G):
                for jj in range(g * NDG, (g + 1) * NDG):
                    j = ND + jj
                    ps = qps.tile([128, SL], F32, tag="qk", bufs=3)
                    for t in range(ND):
                        nc.tensor.matmul(ps[:],
                                         w_qk_sb[t][:, 128 * j:128 * (j + 1)],
                                         ln1xb[t][:], start=(t == 0),
                                         stop=(t == ND - 1))
                    nc.vector.tensor_scalar(k_sb[jj][:], ps[:],
                                            b_qk_sb[:, j:j + 1], None,
                                            op0=ALU.add)
                kview = kv_own[g][0:CB].rearrange("(d s) -> d s", s=SL)
                for t in range(NDG):
                    nc.sync.dma_start(kview[128 * t:128 * (t + 1), :],
                                      k_sb[g * NDG + t][:])
                VCC = DG // ((DG + 511) // 512)
                for m in range(n_s):
                    v_sb = kvp.tile([128, DG], BF16, tag="v_sb", bufs=3,
                                    name=f"v_sb{g}_{m}")
                    for vi in range(DG // VCC):
                        lo = DG * g + VCC * vi
                        psb = qps.tile([128, 512], F32, tag="v", bufs=2)
                        ps = psb[:, 0:VCC]
                        for t in range(ND):
                            nc.tensor.matmul(
                                ps, ln1xb[t][:, 128 * m:128 * (m + 1)],
                                w_v_sb[t][:, lo:lo + VCC],
                                start=(t == 0), stop=(t == ND - 1))
                        nc.vector.tensor_tensor(
                            v_sb[:, VCC * vi:VCC * (vi + 1)], ps,
                            bvb_sb[:, lo:lo + VCC], op=ALU.add)
                    vview = kv_own[g][CB:2 * CB].rearrange("(r d) -> r d",
                                                            d=DG)
                    nc.sync.dma_start(vview[128 * m:128 * (m + 1), :], v_sb[:])
                nc.gpsimd.collective_compute(
                    "AllGather", ALU.bypass, replica_groups=grp,
                    ins=[kv_own[g][:]], outs=[kv_gath[g][:]])

            # q projections last (only needed once attention starts)
            for j in range(ND):
                ps = qps.tile([128, SL], F32, tag="qk", bufs=3)
                for t in range(ND):
                    nc.tensor.matmul(ps[:], w_qk_sb[t][:, 128 * j:128 * (j + 1)],
                                     ln1xb[t][:], start=(t == 0), stop=(t == ND - 1))
                nc.vector.tensor_scalar(q_sb[j][:], ps[:], b_qk_sb[:, j:j + 1],
                                        None, op0=ALU.add)
        p12.release()

        # ==== phase 4: attention =============================================
        with tc.tile_pool(name="attn_sb", bufs=1) as ap, \
             tc.tile_pool(name="sg_ps", bufs=1, space="PSUM") as sps, \
             tc.tile_pool(name="cs_ps", bufs=1, space="PSUM") as cps, \
             tc.tile_pool(name="exp_sb", bufs=1) as epool:
            kT_all = [ap.tile([128, S], BF16, name=f"kT{hp}") for hp in range(HP)]
            v_all = ap.tile([128, NSK * D], BF16)

            HPG = HP // G
            TS = SL // 128
            for g in range(G):
                kvv = kv_gath[g].rearrange("(c h t p w) -> h p c t w",
                                           c=NC, h=2, p=128, w=DG)
                ksrc = kv_gath[g].rearrange("(c h d s) -> h d c s",
                                            c=NC, h=2, s=SL)
                for hh in range(HPG):
                    hp = g * HPG + hh
                    kdst = kT_all[hp].rearrange("p (c s) -> p c s", c=NC)
                    nc.sync.dma_start(kdst[:],
                                      ksrc[0, 128 * hh:128 * (hh + 1), :, :])
                vdst = v_all.rearrange("p (c t hd) -> p c t hd", c=NC,
                                       hd=D)
                for c in range(NC):
                    nc.sync.dma_start(vdst[:, c, :, DG * g:DG * (g + 1)],
                                      kvv[1, :, c, :, :])

            for hp in range(HP):
                ctx = cps.tile([128, SL], F32, tag="ctx", bufs=2, name=f"ctx{hp}")
                sd = cps.tile([33, SL], F32, tag="sd", bufs=2, name=f"sd{hp}")
                pend = None
                for b in range(NSK):
                    sg = sps.tile([128, 2 * SL], F32, tag="sg", bufs=2)
                    nc.tensor.matmul(sg[:, 0:SL],
                                     kT_all[hp][0:64, 128 * b:128 * (b + 1)],
                                     q_sb[hp][0:64, :], start=True, stop=True)
                    nc.tensor.matmul(sg[:, SL:2 * SL],
                                     kT_all[hp][64:128, 128 * b:128 * (b + 1)],
                                     q_sb[hp][64:128, :], start=True, stop=True)
                    ex = epool.tile([128, 2 * SL], BF16, tag="exp", bufs=3)
                    nc.scalar.activation(ex[:], sg[:], AF.Exp, scale=scale)
                    if pend is not None:
                        _pv(nc, cfg, ctx, sd, v_all, ones_cb, hp,
                            pend[0], pend[1])
                    pend = (b, ex)
                _pv(nc, cfg, ctx, sd, v_all, ones_cb, hp, pend[0], pend[1])
                # epilogue: denominators -> srow (+ in-place reciprocal),
                # unnormalized ctx -> SBUF
                strip = srow[:, SL * hp:SL * (hp + 1)]
                nc.vector.tensor_copy(srow[0:1, SL * hp:SL * (hp + 1)],
                                      sd[0:1, :])
                nc.vector.tensor_copy(srow[32:33, SL * hp:SL * (hp + 1)],
                                      sd[32:33, :])
                nc.scalar.activation(strip, strip, AF.Ln)
                nc.scalar.activation(strip, strip, AF.Exp, scale=-1.0)
                nc.vector.tensor_copy(ctxu[hp][0:64, :], ctx[0:64, :])
                nc.vector.tensor_copy(ctxu[hp][64:128, :], ctx[64:128, :])
        x2 = ctxu

        # ==== phase 5+6: normalize + LN2 + FFN ================================
        with tc.tile_pool(name="ffn_sb", bufs=1) as fp:
            w_fc_sb = [fp.tile([128, 4 * D], BF16, name=f"wfc{t}")
                       for t in range(ND)]
            for t in range(ND):
                nc.sync.dma_start(w_fc_sb[t][:], w_fcT[128 * t:128 * (t + 1), :])
            w_pj_sb = [fp.tile([128, D], BF16, name=f"wpj{m}")
                       for m in range(NFF)]
            for m in range(NFF):
                nc.sync.dma_start(w_pj_sb[m][:], w_projT[128 * m:128 * (m + 1), :])
            # softmax normalize + attention residual -> x2 (in ctxu)
            with tc.tile_pool(name="rb_ps", bufs=1, space="PSUM") as rps:
                for hp in range(HP):
                    rb = rps.tile([128, SL], F32, tag="rb", bufs=2,
                                  name=f"rb{hp}")
                    nc.tensor.matmul(rb[0:64, :], ones33[0:1, 0:64],
                                     srow[0:1, SL * hp:SL * (hp + 1)],
                                     start=True, stop=True)
                    nc.tensor.matmul(rb[64:128, :], ones33[32:33, 0:64],
                                     srow[32:33, SL * hp:SL * (hp + 1)],
                                     start=True, stop=True,
                                     tile_position=(32, 64))
                    cn = fp.tile([128, SL], F32, tag="cn", bufs=2,
                                 name=f"cn{hp}")
                    nc.vector.tensor_tensor(cn[:], ctxu[hp][:], rb[:],
                                            op=ALU.mult)
                    nc.vector.tensor_tensor(ctxu[hp][:], cn[:], ln1x[hp][:],
                                            op=ALU.add)
            x2ln = ln1x
            x2lnb = [fp.tile([128, SL], BF16, name=f"x2lnb{t}")
                     for t in range(ND)]
            layernorm_T(x2, ln2w_sb, ln2b_sb, x2ln, x2lnb)
            fps = tc.alloc_tile_pool(name="ffn_ps", bufs=1, space="PSUM")

            h_sb = fp.tile([128, NFF * SL], BF16)
            for m in range(NFF):
                ps = fps.tile([128, SL], F32, tag="h", bufs=4)
                for t in range(ND):
                    nc.tensor.matmul(ps[:], w_fc_sb[t][:, 128 * m:128 * (m + 1)],
                                     x2lnb[t][:], start=(t == 0), stop=(t == ND - 1))
                if not gelu_compose:
                    nc.scalar.activation(h_sb[:, SL * m:SL * (m + 1)], ps[:],
                                         AF.Gelu_apprx_tanh,
                                         bias=b_fc_sb[:, m:m + 1])
                else:
                    c = float(np.sqrt(2.0 / np.pi))
                    hb = fp.tile([128, SL], F32, tag="ghb", bufs=2)
                    t1 = fp.tile([128, SL], F32, tag="gt1", bufs=2)
                    nc.vector.tensor_scalar(hb[:], ps[:], b_fc_sb[:, m:m + 1],
                                            None, op0=ALU.add)
                    nc.vector.tensor_tensor(t1[:], hb[:], hb[:], op=ALU.mult)
                    nc.vector.tensor_scalar(t1[:], t1[:], 0.044715 * c, c,
                                            op0=ALU.mult, op1=ALU.add)
                    nc.vector.tensor_tensor(t1[:], hb[:], t1[:], op=ALU.mult)
                    nc.scalar.activation(t1[:], t1[:], AF.Tanh)
                    nc.vector.tensor_scalar(t1[:], t1[:], 0.5, 0.5,
                                            op0=ALU.mult, op1=ALU.add)
                    nc.vector.tensor_tensor(h_sb[:, SL * m:SL * (m + 1)],
                                            hb[:], t1[:], op=ALU.mult)
            for t in range(ND):
                ps = fps.tile([128, SL], F32, tag="o", bufs=2)
                for m in range(NFF):
                    nc.tensor.matmul(ps[:], w_pj_sb[m][:, 128 * t:128 * (t + 1)],
                                     h_sb[:, SL * m:SL * (m + 1)],
                                     start=(m == 0), stop=(m == NFF - 1))
                fsum = fp.tile([128, SL], F32, tag="fsum", bufs=2, name=f"fs{t}")
                nc.vector.tensor_scalar(fsum[:], ps[:], b_proj_sb[:, t:t + 1],
                                        None, op0=ALU.add)
                o = fp.tile([128, SL], F32, tag="out", bufs=2, name=f"o{t}")
                nc.vector.tensor_tensor(o[:], fsum[:], x2ln[t][:], op=ALU.add)
                nc.sync.dma_start(outT[128 * t:128 * (t + 1), :], o[:])
            fps.release()

    nc.compile()
    return nc


def _pv(nc, cfg, ctx, sd, v_all, ones_cb, hp, b, ex):
    SL, D, NSK = cfg.SL, cfg.D, cfg.NSK
    st, sp = (b == 0), (b == NSK - 1)
    nc.tensor.matmul(ctx[0:64, :], v_all[:, b * D + 128 * hp:b * D + 128 * hp + 64],
                     ex[:, 0:SL], start=st, stop=sp, skip_group_check=True)
    nc.tensor.matmul(ctx[64:128, :],
                     v_all[:, b * D + 128 * hp + 64:b * D + 128 * (hp + 1)],
                     ex[:, SL:2 * SL], start=st, stop=sp, skip_group_check=True,
                     tile_position=(0, 64))
    nc.tensor.matmul(sd[0:1, :], ones_cb[:], ex[:, 0:SL], start=st, stop=sp,
                     skip_group_check=True)
    nc.tensor.matmul(sd[32:33, :], ones_cb[:], ex[:, SL:2 * SL], start=st,
                     stop=sp, skip_group_check=True, tile_position=(0, 32))


# ---- host side --------------------------------------------------------------

def _prep_inputs(cfg, x, ln1_w, ln1_b, w_attn, b_attn, ln2_w, ln2_b,
                 w_fc, b_fc, w_proj, b_proj):
    D, H, NC, SL, ND, NFF = cfg.D, cfg.H, cfg.NC, cfg.SL, cfg.ND, cfg.NFF
    import ml_dtypes
    bf16 = ml_dtypes.bfloat16

    def pp(v, n):  # per-partition layout [128, n]
        return np.ascontiguousarray(v.reshape(n, 128).T.astype(np.float32))

    common = {
        "w_qkT": np.ascontiguousarray(w_attn[:2 * D].T.astype(bf16)),
        "w_vT": np.ascontiguousarray(w_attn[2 * D:].T.astype(bf16)),
        "b_qk": pp(b_attn[:2 * D], 2 * D // 128),
        "b_v": np.ascontiguousarray(b_attn[2 * D:].reshape(1, D).astype(np.float32)),
        "ln1w": pp(ln1_w, ND), "ln1b": pp(ln1_b, ND),
        "ln2w": pp(ln2_w, ND), "ln2b": pp(ln2_b, ND),
        "w_fcT": np.ascontiguousarray(w_fc.T.astype(bf16)),
        "b_fc": pp(b_fc, NFF),
        "w_projT": np.ascontiguousarray(w_proj.T.astype(bf16)),
        "b_proj": pp(b_proj, ND),
    }
    xT = np.ascontiguousarray(x.T.astype(np.float32))
    in_maps = []
    for c in range(NC):
        m = dict(common)
        m["xT"] = np.ascontiguousarray(xT[:, c * SL:(c + 1) * SL])
        in_maps.append(m)
    return in_maps


_CACHE = {}


def kernel(**inputs):
    cfg = Cfg()
    inputs = {k: np.asarray(v) for k, v in inputs.items()}
    in_maps = _prep_inputs(cfg, **inputs)
    if "nc" not in _CACHE:
        _CACHE["nc"] = build(cfg)
    nc = _CACHE["nc"]
    from concourse.bass_utils import run_bass_kernel_spmd
    res = run_bass_kernel_spmd(nc, in_maps, list(range(cfg.NC)))
    outs = [np.asarray(res.results[c]["outT"], dtype=np.float32).T
            for c in range(cfg.NC)]
    return np.ascontiguousarray(np.concatenate(outs, axis=0))
